# revision 67
# baseline (speedup 1.0000x reference)
"""Trainium2 Bass kernel: Longformer-style windowed attention with rotary,
head-averaged K/V (step_attn), fused QKV/out projections.

Sharding: 8 cores = (batch 2) x (sequence-quarter 4). Each core computes its
512 output rows for all 16 heads. No collectives: the windowed attention for
a 512-row quarter only needs 6 key-tiles (128 rows each) of the head-averaged
K/V plus the 64 global-token rows, all of which the core computes itself from
host-sliced hidden-state rows. Head-averaging of K/V commutes with rotary and
with the (linear) projection, so the K/V-mean projection weights are folded on
host to [2048, 256].

fp8 strategy (DoubleRow matmuls, 2 k-tiles of 128 per instruction at 0.5
cycles/row): error-attenuated paths (Q proj, K proj, QK scores) run naive fp8;
the V projection is hi+lo compensated (residual term via one DoubleRow matmul
per k-tile pairing hid_lo*w_hi + hi*lo); attention-value matmul, softmax sums
and out-projection stay bf16. Weights are host-scaled by 64 to clear the fp8
subnormal range; the 1/64 folds into downstream constant scales.
"""

import sys

for _p in ("/opt/trn_rl_repo", "/root/.axon_site/_ro/trn_rl_repo"):
    if _p not in sys.path:
        sys.path.append(_p)

import numpy as np
import ml_dtypes

import concourse.bass as bass
import concourse.tile as tile
from concourse import bacc
from concourse import bass_isa
import concourse.mybir as mybir
from concourse.bass_utils import run_bass_kernel_spmd

F32 = mybir.dt.float32
BF16 = mybir.dt.bfloat16
F8 = mybir.dt.float8e4
F8NP = ml_dtypes.float8_e4m3
DR = mybir.MatmulPerfMode.DoubleRow
MUL = mybir.AluOpType.mult
ADD = mybir.AluOpType.add
SUB = mybir.AluOpType.subtract
COPY = mybir.ActivationFunctionType.Copy
EXP = mybir.ActivationFunctionType.Exp

H = 16
D = 128
ROT = 32
HALF = 16  # ROT // 2
WIN = 256
G = 64
BASE = 10000.0
S = 2048
HD = H * D
B = 2
NCORES = 8
QROWS = 512          # rows per core
NKV = 6              # kv key-tiles per core
KVG_ROWS = NKV * 128 + G  # 832
SCALE = 1.0 / float(np.sqrt(np.float32(D)))
WS = 64.0            # host weight scale (fp8 subnormal avoidance)


# ---------------------------------------------------------------- device ----

def build_nc():
    nc = bacc.Bacc("TRN2", target_bir_lowering=False, debug=False,
                   num_devices=NCORES)

    aps = {}
    def inp(name, shape, dt):
        aps[name] = nc.dram_tensor(name, shape, dt, kind="ExternalInput").ap()

    # hidT8: transposed hidden states, fp8, planes (lo, hi) per k-tile
    inp("hidT8", [128, 16, 2, KVG_ROWS], F8)
    inp("wq8", [128, 16, HD], F8)            # 64*Wq, feature-major, naive
    inp("wkv8", [128, 16, 2, 2 * D], F8)     # 64*Wkv, planes (hi, lo16)
    inp("wo", [HD, HD], BF16)
    inp("bqb", [128, HD], BF16)              # 64*b_q broadcast to partitions
    inp("bob", [128, HD], BF16)              # b_o broadcast to partitions
    inp("pk128", [128, 8 * HALF + 2 * NKV * HALF + NKV], F32)
    inp("pk64", [G, 2 * HALF], F32)
    inp("pkb", [1, 2 * D + HD], BF16)        # 64*b_kv | b_o
    inp("i8", [128, 128], F8)                # 240 * identity (mask carrier)
    inp("ib16", [128, 128], BF16)            # identity (PE transposes)
    inp("mask8", [128, 16, 512], F8)         # -240*!valid: (3 win t + glob)x4h
    aps["out"] = nc.dram_tensor("out", [QROWS, HD], BF16,
                                kind="ExternalOutput").ap()

    with tile.TileContext(nc) as tc:
        _build_tile(nc, tc, aps)
    nc.compile()
    return nc


def _build_tile(nc, tc, aps):
    from contextlib import ExitStack
    import os
    ctx = ExitStack()
    _PH = int(os.environ.get("KERNEL_PHASES", "4"))

    persist = ctx.enter_context(tc.tile_pool(name="persist", bufs=1))
    ps = ctx.enter_context(tc.tile_pool(name="ps", bufs=6, space="PSUM"))
    # right-side pools released after the q projection
    ctxR = ExitStack()
    hidp = ctxR.enter_context(tc.tile_pool(name="hidp", bufs=1, side="right"))
    wpool = ctxR.enter_context(tc.tile_pool(name="wstream", bufs=8, side="right"))
    epool = ctxR.enter_context(tc.tile_pool(name="evac", bufs=2, side="right"))

    # ---------------- persistent tiles
    hidT8 = hidp.tile([128, 16, 2, KVG_ROWS], F8, tag="hidT8")
    bqb = hidp.tile([128, HD], BF16, tag="bqb")
    bob = persist.tile([128, HD], BF16, tag="bob")
    q_sb = persist.tile([128, 4, HD], BF16, tag="q_sb")
    # q8: fp8 q (16 head blocks) + 16 additive-mask blocks (DoubleRow halves)
    q8 = persist.tile([128, 32, QROWS], F8, tag="q8")
    kv_sb = persist.tile([128, NKV, 2 * D], BF16, tag="kv_sb")
    kvg_sb = persist.tile([G, 2 * D], BF16, tag="kvg_sb")
    kTm = persist.tile([128, NKV, 2, 128], F8, tag="kTm")
    kgTm = persist.tile([128, 2, G], F8, tag="kgTm")
    wkv8 = persist.tile([128, 16, 2, 2 * D], F8, tag="wkv8")
    wo_sb = persist.tile([128, H, HD], BF16, tag="wo_sb")
    i8_sb = persist.tile([128, 128], F8, tag="i8")
    ib16 = persist.tile([128, 128], BF16, tag="ib16")
    ones_c64 = persist.tile([128, 1], BF16, tag="ones_c64")  # 64.0 column
    ones_r = persist.tile([1, 128], BF16, tag="ones_r")   # row (K=1, M=128)
    pk128 = persist.tile([128, 8 * HALF + 2 * NKV * HALF + NKV], F32,
                         tag="pk128")
    pk64 = persist.tile([G, 2 * HALF], F32, tag="pk64")
    pkb = persist.tile([1, 2 * D + HD], BF16, tag="pkb")
    cq_sb = pk128[:, 0:64].rearrange("p (so r) -> p so r", r=HALF)
    sq_sb = pk128[:, 64:128].rearrange("p (so r) -> p so r", r=HALF)
    ckv_sb = pk128[:, 128:224].rearrange("p (t r) -> p t r", r=HALF)
    skv_sb = pk128[:, 224:320].rearrange("p (t r) -> p t r", r=HALF)
    am_sb = pk128[:, 320:326]
    cg_sb = pk64[:, 0:HALF]
    sg_sb = pk64[:, HALF:2 * HALF]
    bkv_sb = pkb[:, 0:2 * D]
    bo_sb = pkb[:, 2 * D:2 * D + HD]

    # ---------------- small loads (Activation HWDGE queue, ordered by need)
    nc.gpsimd.memset(ones_c64[:], 64.0)
    nc.gpsimd.memset(ones_r[:], 1.0)
    for nm, t in (("pk128", pk128), ("pk64", pk64), ("pkb", pkb),
                  ("ib16", ib16), ("bqb", bqb), ("i8", i8_sb),
                  ("bob", bob), ("wkv8", wkv8)):
        nc.scalar.dma_start(out=t[:], in_=aps[nm])
    # additive mask blocks (g = 4..7 of q8's 8 groups)
    nc.scalar.dma_start(out=q8[:, 16:32, :], in_=aps["mask8"])
    # hidT8 lo planes on the Activation queue (KV-proj correction, ~30us in)
    for k2 in range(8):
        nc.scalar.dma_start(out=hidT8[:, 2 * k2:2 * k2 + 2, 0, :],
                            in_=aps["hidT8"][:, 2 * k2:2 * k2 + 2, 0, :])
    # identity carriers into the DoubleRow second halves of kTm / kgTm
    nc.vector.tensor_copy(
        kTm[:, :, 1, :],
        i8_sb[:].rearrange("p (o d) -> p o d", o=1).to_broadcast([128, NKV, 128]))
    nc.vector.tensor_copy(kgTm[:, 1, :], i8_sb[:, 0:G])

    # rotary (in-place, f32 temps): x1' = x1*c - x2*s ; x2' = x2*c + x1*s
    def rotary(x1, x2, c, s, shape, tag, eng=None):
        eng = eng or nc.vector
        t1 = epool.tile(shape, F32, tag=tag + "1")
        t2 = epool.tile(shape, F32, tag=tag + "2")
        eng.tensor_tensor(out=t1[:], in0=x1, in1=s, op=MUL)
        eng.tensor_tensor(out=t2[:], in0=x2, in1=s, op=MUL)
        eng.tensor_tensor(out=x1, in0=x1, in1=c, op=MUL)
        eng.tensor_tensor(out=x1, in0=x1, in1=t2[:], op=SUB)
        eng.tensor_tensor(out=x2, in0=x2, in1=c, op=MUL)
        eng.tensor_tensor(out=x2, in0=x2, in1=t1[:], op=ADD)

    # ---------------- q projection -> q_sb [128p, 4so, 2048c] (64-scaled)
    # processed in two so-halves so full-width wq8 chunks (large DMA
    # descriptors) fit 8 PSUM banks; chunks stay in SBUF for the 2nd half
    if _PH < 2:
        ctxR.close()
        ctx.close()
        return
    wq_ts = []

    def qp_so(so):
        pq = [ps.tile([128, 512], F32, tag="ps", name=f"pq{so}_{i}")
              for i in range(4)]
        for k2 in range(8):
            if so == 0:
                nc.sync.dma_start(out=hidT8[:, 2 * k2:2 * k2 + 2, 1, :],
                                  in_=aps["hidT8"][:, 2 * k2:2 * k2 + 2, 1, :])
                wq_t = wpool.tile([128, 2, HD], F8, tag="wq_t",
                                  name=f"wq_t{k2}")
                nc.sync.dma_start(out=wq_t[:],
                                  in_=aps["wq8"][:, 2 * k2:2 * k2 + 2, :])
                wq_ts.append(wq_t)
            for cc in range(4):
                nc.tensor.matmul(pq[cc][:],
                                 hidT8[:, 2 * k2:2 * k2 + 2, 1,
                                       256 + so * 128:384 + so * 128],
                                 wq_ts[k2][:, :, cc * 512:(cc + 1) * 512],
                                 start=(k2 == 0), stop=(k2 == 7),
                                 perf_mode=DR)
        for cc in range(4):
            nc.vector.tensor_tensor(
                out=q_sb[:, so, cc * 512:(cc + 1) * 512],
                in0=pq[cc][:],
                in1=bqb[:, cc * 512:(cc + 1) * 512], op=ADD)
        # rotary (odd so on gpsimd, even on DVE: parallel streams),
        # then on-chip PE transposes into the fp8 q8 blocks
        qv = q_sb[:, so, :].rearrange("p (h d) -> p h d", d=D)
        c = cq_sb[:, so:so + 1, :].to_broadcast([128, H, HALF])
        s = sq_sb[:, so:so + 1, :].to_broadcast([128, H, HALF])
        rotary(qv[:, :, 0:HALF], qv[:, :, HALF:2 * HALF], c, s,
               [128, H, HALF], "rq",
               eng=(nc.vector if so % 2 == 0 else nc.gpsimd))
        for hh in range(2):
            ptf = ps.tile([128, 8, 128], BF16, tag="pst", bufs=2,
                          name=f"ptq{so}_{hh}")
            pt = ptf[:]
            for h in range(8):
                nc.tensor.transpose(
                    pt[:, h, :],
                    q_sb[:, so, (8 * hh + h) * 128:(8 * hh + h + 1) * 128],
                    ib16[:])
            if hh == 0:
                nc.scalar.activation(
                    q8[:, 8 * hh:8 * hh + 8, so * 128:(so + 1) * 128],
                    pt, COPY, scale=1.0 / WS)
            else:
                nc.vector.tensor_scalar_mul(
                    q8[:, 8 * hh:8 * hh + 8, so * 128:(so + 1) * 128],
                    pt, 1.0 / WS)

    qp_so(0)
    qp_so(1)

    # ---------------- kv projection (+ glob rows); k naive, v compensated
    # emitted between q-projection halves so the k-chain (rotary, PE
    # transposes, fp8 copies) overlaps the DMA-gated q stream
    for st in range(NKV + 1):
        m = 128 if st < NKV else G
        pkv = ps.tile([128, 512], F32, tag="ps")     # main: 64*(k|v)
        pkc = ps.tile([128, 512], F32, tag="ps")     # corr: 16*64*v residue
        for k2 in range(8):
            nc.tensor.matmul(pkv[:m, :2 * D],
                             hidT8[:, 2 * k2:2 * k2 + 2, 1, st * 128:st * 128 + m],
                             wkv8[:, 2 * k2:2 * k2 + 2, 0, :],
                             start=(k2 == 0), stop=False, perf_mode=DR)
        nc.tensor.matmul(pkv[:m, :2 * D], ones_r[:, :m], bkv_sb[:],
                         start=False, stop=True)
        for kt in range(16):
            nc.tensor.matmul(pkc[:m, :D],
                             hidT8[:, kt, :, st * 128:st * 128 + m],
                             wkv8[:, kt, :, D:2 * D],
                             start=(kt == 0), stop=(kt == 15), perf_mode=DR)
        dst = kv_sb[:, st, :] if st < NKV else kvg_sb[:]
        vcorr = epool.tile([128, D], F32, tag="vcorr")
        nc.scalar.activation(vcorr[:m], pkc[:m, :D], COPY, scale=1.0 / 16.0)
        nc.scalar.copy(dst[:m, 0:D], pkv[:m, 0:D])
        nc.vector.tensor_tensor(out=dst[:m, D:2 * D], in0=pkv[:m, D:2 * D],
                                in1=vcorr[:m], op=ADD)

    rotary(kv_sb[:, :, 0:HALF], kv_sb[:, :, HALF:2 * HALF],
           ckv_sb[:], skv_sb[:], [128, NKV, HALF], "rkv", eng=nc.gpsimd)
    rotary(kvg_sb[:, 0:HALF], kvg_sb[:, HALF:2 * HALF],
           cg_sb[:], sg_sb[:], [G, HALF], "rg", eng=nc.gpsimd)

    # kT via PE transposes straight into the fp8 DoubleRow layout
    ptkf = ps.tile([128, 8, 128], BF16, tag="pst", bufs=2, name="ptk")
    ptk = ptkf[:]
    for st in range(NKV):
        nc.tensor.transpose(ptk[:, st, :], kv_sb[:, st, 0:128], ib16[:])
    nc.scalar.activation(kTm[:, :, 0, :], ptk[:, 0:NKV, :], COPY,
                         scale=1.0 / WS)
    ptgf = ps.tile([128, 8, 128], BF16, tag="pst", bufs=2, name="ptg")
    ptg = ptgf[:, 0, :]
    nc.tensor.transpose(ptg[:, 0:G], kvg_sb[:, 0:128], ib16[0:G, 0:G])
    nc.scalar.activation(kgTm[:, 0, :], ptg[:, 0:G], COPY, scale=1.0 / WS)

    qp_so(2)
    qp_so(3)

    # wo loads: gated on q8 readiness so the DMA engines stay clear for the
    # projection-phase critical path; they stream during early attention.
    for hg in range(4):
        nc.gpsimd.tensor_copy(wo_sb[0:1, 0, hg * 512:hg * 512 + 1],
                              q8[0:1, 4 * hg, 0:1])
        nc.sync.dma_start(
            out=wo_sb[:, :, hg * 512:(hg + 1) * 512],
            in_=aps["wo"].rearrange("(h p) n -> p h n", p=128)
                [:, :, hg * 512:(hg + 1) * 512])

    ctxR.close()

    # ---------------- fused attention + out-projection, per block L
    wexp = ctx.enter_context(tc.tile_pool(name="wexp", bufs=8))
    rzp = ctx.enter_context(tc.tile_pool(name="rzp", bufs=4))
    opool = ctx.enter_context(tc.tile_pool(name="opool", bufs=2))
    if _PH < 3:
        ctx.close()
        return
    aT_tiles = [None] * 4
    aT8_tiles = [None] * 4
    # g-blocks of q8: 0-3 = q head groups, 4-6 = win masks t, 7 = glob mask
    q8v = q8[:].rearrange("p (g h) r -> p g h r", h=4)

    def scores(L, hg):
        """Emit the 4 DoubleRow score matmuls (half1 carries additive mask)."""
        st = {}
        for t in range(3):
            rhs = q8v[:, hg:5 + t:(4 + t - hg), :, L * 128:(L + 1) * 128]
            p_t = ps.tile([128, 512], F32, tag="ps", name=f"p_t{L}_{hg}_{t}")
            nc.tensor.matmul(p_t[:], kTm[:, L + t, :, :], rhs,
                             start=True, stop=True, perf_mode=DR)
            st[t] = p_t
        rhs = q8v[:, hg:8:(7 - hg), :, L * 128:(L + 1) * 128]
        p_g = ps.tile([128, 512], F32, tag="ps", name=f"p_g{L}_{hg}")
        nc.tensor.matmul(p_g[:G, :], kgTm[:], rhs, start=True, stop=True,
                         perf_mode=DR)
        st["g"] = p_g
        return st

    def post(L, hg, st, aT):
        """exp -> softmax sums -> AV -> normalized aT for (L, hg)."""
        w_t = []
        for t in range(3):
            w = wexp.tile([128, 512], BF16, tag="wexp", name=f"w{L}_{hg}_{t}")
            nc.scalar.activation(w[:], st[t][:], EXP,
                                 bias=am_sb[:, L + t:L + t + 1], scale=SCALE)
            w_t.append(w)
        w_g = wexp.tile([G, 512], BF16, tag="wexpg", bufs=4,
                        name=f"wg{L}_{hg}")
        nc.scalar.activation(w_g[:], st["g"][:G, :], EXP, scale=SCALE)

        # softmax denominator: 64z via ones-matmuls, broadcast, reciprocal
        # (the 64 cancels po's 64-scale in the aT multiply)
        pz = ps.tile([128, 512], F32, tag="ps", name=f"pz{L}_{hg}")
        for t in range(3):
            nc.tensor.matmul(pz[:1, :], ones_c64[:], w_t[t][:],
                             start=(t == 0), stop=False)
        nc.tensor.matmul(pz[:1, :], ones_c64[:G, :], w_g[:],
                         start=False, stop=True)
        zh = rzp.tile([1, 512], BF16, tag="z_sb", name=f"z{L}_{hg}")
        nc.vector.tensor_copy(zh[:], pz[:1, :])

        po = ps.tile([128, 512], F32, tag="ps", name=f"po{L}_{hg}")
        for t in range(3):
            nc.tensor.matmul(po[:], kv_sb[:, L + t, D:2 * D], w_t[t][:],
                             start=(t == 0), stop=False)
        nc.tensor.matmul(po[:], kvg_sb[:, D:2 * D], w_g[:],
                         start=False, stop=True)

        pzb = ps.tile([128, 512], F32, tag="ps", name=f"pzb{L}_{hg}")
        nc.tensor.matmul(pzb[:], ones_r[:], zh[:], start=True, stop=True)
        rzb = rzp.tile([128, 512], F32, tag="rzb", name=f"rzb{L}_{hg}")
        nc.vector.reciprocal(out=rzb[:], in_=pzb[:])
        nc.vector.tensor_tensor(
            out=aT[:, 4 * hg:4 * hg + 4, :],
            in0=po[:].rearrange("p (h s) -> p h s", s=128),
            in1=rzb[:].rearrange("p (h s) -> p h s", s=128),
            op=MUL)

    # out-projection for block Lp, one output-column chunk ncn per call
    oproj_state = {}

    def oproj_seg(Lp, ncn):
        aT = aT_tiles[Lp]
        po2 = ps.tile([128, 512], F32, tag="ps", name=f"po2_{Lp}_{ncn}")
        for h in range(16):
            nc.tensor.matmul(po2[:], aT[:, h, :],
                             wo_sb[:, h, ncn * 512:(ncn + 1) * 512],
                             start=(h == 0), stop=(h == 15))
        o_sb = oproj_state.setdefault(
            Lp, opool.tile([128, HD], BF16, tag="o_sb", name=f"o_sb{Lp}"))
        nc.vector.tensor_tensor(
            out=o_sb[:, ncn * 512:(ncn + 1) * 512], in0=po2[:],
            in1=bob[:, ncn * 512:(ncn + 1) * 512], op=ADD)
        if ncn == 3:
            nc.sync.dma_start(out=aps["out"][Lp * 128:(Lp + 1) * 128, :],
                              in_=o_sb[:])

    # software pipeline: scores(hg+1) issued before post(hg); out-proj of
    # block L-1 interleaved between posts. wo streams in n-column chunks on
    # the sync queue during attention L0 (keeps it off the startup DMA path).
    for L in range(4):
        aT_tiles[L] = wexp.tile([128, H, 128], BF16, tag="aT", bufs=2,
                                name=f"aT{L}")
        st_prev = scores(L, 0)
        for hg in range(4):
            st_next = scores(L, hg + 1) if hg < 3 else None
            post(L, hg, st_prev, aT_tiles[L])
            st_prev = st_next
            if _PH >= 4 and L >= 1:
                oproj_seg(L - 1, hg)
    if _PH >= 4:
        for ncn in range(4):
            oproj_seg(3, ncn)

    ctx.close()


# ------------------------------------------------------------------ host ----

_NC_CACHE = None


def _get_nc():
    global _NC_CACHE
    if _NC_CACHE is None:
        _NC_CACHE = build_nc()
    return _NC_CACHE


def _f8(x):
    return np.asarray(x, np.float32).astype(F8NP)


def _f8_pair(x):
    """(hi, lo) with x ~= hi + lo/16, both fp8."""
    hi = _f8(x)
    lo = _f8((np.asarray(x, np.float32) - hi.astype(np.float32)) * 16.0)
    return hi, lo


def make_in_maps(hidden_states, attention_mask, glob_idx, W_qkv, b_qkv, W_o, b_o):
    bf = ml_dtypes.bfloat16
    hidden_states = np.asarray(hidden_states, np.float32)
    attention_mask = np.asarray(attention_mask, np.float32)
    glob_idx = np.asarray(glob_idx)
    W_qkv = np.asarray(W_qkv, np.float32)
    b_qkv = np.asarray(b_qkv, np.float32)
    W_o = np.asarray(W_o, np.float32)
    b_o = np.asarray(b_o, np.float32)

    w3 = W_qkv.reshape(HD, H, 3 * D)
    wq = np.ascontiguousarray(w3[:, :, :D].reshape(HD, HD))
    wkv = np.concatenate([w3[:, :, D:2 * D].mean(axis=1),
                          w3[:, :, 2 * D:].mean(axis=1)], axis=1)
    # feature-major fp8 layouts, 64-scaled
    wq8 = _f8((WS * wq).reshape(16, 128, HD).transpose(1, 0, 2))
    wkv_hi, wkv_lo = _f8_pair((WS * wkv).reshape(16, 128, 2 * D))
    wkv8 = np.stack([wkv_hi, wkv_lo], axis=2).transpose(1, 0, 2, 3)
    wkv8 = np.ascontiguousarray(wkv8)      # [128, 16, 2(hi,lo), 256]

    b3 = b_qkv.reshape(H, 3 * D)
    bq = np.ascontiguousarray(b3[:, :D].reshape(1, HD)).astype(np.float32)
    bqb = np.ascontiguousarray(np.broadcast_to(WS * bq, (128, HD))).astype(bf)
    bob = np.ascontiguousarray(np.broadcast_to(b_o[None, :], (128, HD))
                               ).astype(bf)
    bkv = np.concatenate([b3[:, D:2 * D].mean(axis=0),
                          b3[:, 2 * D:].mean(axis=0)])[None, :] * WS
    bo = b_o[None, :]
    pkb = np.concatenate([bkv, bo], axis=1).astype(bf)
    wo = W_o.astype(bf)

    inv_freq = 1.0 / (BASE ** (np.arange(0, ROT, 2, dtype=np.float32) / ROT))
    freqs = np.arange(S, dtype=np.float32)[:, None] * inv_freq[None, :]  # [S,16]
    cos_all = np.cos(freqs).astype(np.float32)
    sin_all = np.sin(freqs).astype(np.float32)

    in_maps = []
    for c in range(NCORES):
        b, q = divmod(c, 4)
        t0 = 4 * q - 2
        tiles = [max(0, t0 + i) for i in range(NKV)]       # clipped content
        intended = [t0 + i for i in range(NKV)]
        kv_rows = np.concatenate([np.arange(t * 128, t * 128 + 128)
                                  for t in tiles])
        g_rows = glob_idx[b].astype(np.int64)
        rows = np.concatenate([kv_rows, g_rows])
        hid_c = np.ascontiguousarray(hidden_states[b][rows])   # [832, 2048]
        # transpose to [128 fsub, 16 ftile, rows], fp8 (lo, hi) planes
        hidT = hid_c.T.reshape(16, 128, KVG_ROWS).transpose(1, 0, 2)
        h_hi, h_lo = _f8_pair(hidT)
        hidT8 = np.ascontiguousarray(np.stack([h_lo, h_hi], axis=2))

        q_rows = np.arange(QROWS * q, QROWS * (q + 1))
        cos_q = cos_all[q_rows].reshape(4, 128, HALF).transpose(1, 0, 2).copy()
        sin_q = sin_all[q_rows].reshape(4, 128, HALF).transpose(1, 0, 2).copy()
        cos_kv = cos_all[kv_rows].reshape(NKV, 128, HALF).transpose(1, 0, 2).copy()
        sin_kv = sin_all[kv_rows].reshape(NKV, 128, HALF).transpose(1, 0, 2).copy()
        cos_g = cos_all[g_rows].copy()
        sin_g = sin_all[g_rows].copy()

        am = attention_mask[b, 0, 0]                        # [S]
        am_loc = am[kv_rows].reshape(NKV, 128).T.copy()     # [128, NKV]

        # additive fp8 masks, folded into the DoubleRow score matmuls:
        # score += 240 * m8 where m8 = -240 for invalid -> -57600 pre-scale.
        # layout [key-p, block(t0,t1,t2,glob), 4h replicated, 4L*128 rows]
        mask8 = np.full((128, 4, 4, 512), -240.0, np.float32)
        for L in range(4):
            rows_glb = QROWS * q + L * 128 + np.arange(128)
            for t in range(3):
                it = intended[L + t]
                if it < 0:
                    continue
                key_pos = it * 128 + np.arange(128)
                valid = (key_pos[:, None] <= rows_glb[None, :]) & \
                        (key_pos[:, None] >= rows_glb[None, :] - (WIN - 1))
                mask8[:, t, :, L * 128:(L + 1) * 128] = \
                    np.where(valid, 0.0, -240.0)[:, None, :]
        # glob (block 3): row >= WIN and glob_idx < row - WIN; key rows 64-127
        # are dead (identity carrier is zero there)
        rows_glb = QROWS * q + np.arange(QROWS)
        validg = ((rows_glb[None, :] >= WIN) &
                  (g_rows[:, None] < rows_glb[None, :] - WIN))
        mask8[:G, 3, :, :] = np.where(validg, 0.0, -240.0)[:, None, :]
        mask8 = mask8.reshape(128, 16, 512)

        i8 = (240.0 * np.eye(128, dtype=np.float32))
        ib16 = np.eye(128, dtype=np.float32).astype(bf)

        pk128 = np.concatenate(
            [cos_q.reshape(128, 64), sin_q.reshape(128, 64),
             cos_kv.reshape(128, 96), sin_kv.reshape(128, 96),
             am_loc], axis=1).astype(np.float32)
        pk64 = np.concatenate([cos_g, sin_g], axis=1).astype(np.float32)
        in_maps.append({
            "hidT8": hidT8,
            "wq8": wq8, "wkv8": wkv8, "wo": wo,
            "bqb": bqb, "bob": bob,
            "pk128": pk128, "pk64": pk64, "pkb": pkb,
            "i8": _f8(i8), "ib16": ib16, "mask8": _f8(mask8),
        })
    return in_maps


def kernel(hidden_states, attention_mask, glob_idx, W_qkv, b_qkv, W_o, b_o):
    nc = _get_nc()
    in_maps = make_in_maps(hidden_states, attention_mask, glob_idx,
                           W_qkv, b_qkv, W_o, b_o)
    res = run_bass_kernel_spmd(nc, in_maps, core_ids=list(range(NCORES)))
    out = np.empty((B, S, HD), np.float32)
    for c in range(NCORES):
        b, q = divmod(c, 4)
        out[b, QROWS * q:QROWS * (q + 1), :] = \
            res.results[c]["out"].astype(np.float32)
    return out


# revision 68
# speedup vs baseline: 1.0302x; 1.0302x over previous
"""Trainium2 Bass kernel: Longformer-style windowed attention with rotary,
head-averaged K/V (step_attn), fused QKV/out projections.

Sharding: 8 cores = (batch 2) x (sequence-quarter 4). Each core computes its
512 output rows for all 16 heads. No collectives: the windowed attention for
a 512-row quarter only needs 6 key-tiles (128 rows each) of the head-averaged
K/V plus the 64 global-token rows, all of which the core computes itself from
host-sliced hidden-state rows. Head-averaging of K/V commutes with rotary and
with the (linear) projection, so the K/V-mean projection weights are folded on
host to [2048, 256].

fp8 strategy (DoubleRow matmuls, 2 k-tiles of 128 per instruction at 0.5
cycles/row): error-attenuated paths (Q proj, K proj, QK scores) run naive fp8;
the V projection is hi+lo compensated (residual term via one DoubleRow matmul
per k-tile pairing hid_lo*w_hi + hi*lo); attention-value matmul, softmax sums
and out-projection stay bf16. Weights are host-scaled by 64 to clear the fp8
subnormal range; the 1/64 folds into downstream constant scales.
"""

import sys

for _p in ("/opt/trn_rl_repo", "/root/.axon_site/_ro/trn_rl_repo"):
    if _p not in sys.path:
        sys.path.append(_p)

import numpy as np
import ml_dtypes

import concourse.bass as bass
import concourse.tile as tile
from concourse import bacc
from concourse import bass_isa
import concourse.mybir as mybir
from concourse.bass_utils import run_bass_kernel_spmd

F32 = mybir.dt.float32
BF16 = mybir.dt.bfloat16
F8 = mybir.dt.float8e4
F8NP = ml_dtypes.float8_e4m3
DR = mybir.MatmulPerfMode.DoubleRow
MUL = mybir.AluOpType.mult
ADD = mybir.AluOpType.add
SUB = mybir.AluOpType.subtract
COPY = mybir.ActivationFunctionType.Copy
EXP = mybir.ActivationFunctionType.Exp

H = 16
D = 128
ROT = 32
HALF = 16  # ROT // 2
WIN = 256
G = 64
BASE = 10000.0
S = 2048
HD = H * D
B = 2
NCORES = 8
QROWS = 512          # rows per core
NKV = 6              # kv key-tiles per core
KVG_ROWS = NKV * 128 + G  # 832
SCALE = 1.0 / float(np.sqrt(np.float32(D)))
WS = 64.0            # host weight scale (fp8 subnormal avoidance)


# ---------------------------------------------------------------- device ----

def build_nc():
    nc = bacc.Bacc("TRN2", target_bir_lowering=False, debug=False,
                   num_devices=NCORES)

    aps = {}
    def inp(name, shape, dt):
        aps[name] = nc.dram_tensor(name, shape, dt, kind="ExternalInput").ap()

    # hidT8: transposed hidden states, fp8, planes (lo, hi) per k-tile
    inp("hidT8", [128, 16, 2, KVG_ROWS], F8)
    inp("wq8", [128, 16, HD], F8)            # 64*Wq, feature-major, naive
    inp("wkv8", [128, 16, 2, 2 * D], F8)     # 64*Wkv, planes (hi, lo16)
    inp("wo", [HD, HD], BF16)
    inp("bqb", [128, HD], BF16)              # 64*b_q broadcast to partitions
    inp("bob", [128, HD], BF16)              # b_o broadcast to partitions
    inp("pk128", [128, 8 * HALF + 2 * NKV * HALF + NKV], F32)
    inp("pk64", [G, 2 * HALF], F32)
    inp("pkb", [1, 2 * D + HD], BF16)        # 64*b_kv | b_o
    inp("i8", [128, 128], F8)                # 240 * identity (mask carrier)
    inp("ib16", [128, 128], BF16)            # identity (PE transposes)
    inp("mask8", [128, 16, 512], F8)         # -240*!valid: (3 win t + glob)x4h
    aps["out"] = nc.dram_tensor("out", [QROWS, HD], BF16,
                                kind="ExternalOutput").ap()

    with tile.TileContext(nc) as tc:
        _build_tile(nc, tc, aps)
    nc.compile()
    return nc


def _build_tile(nc, tc, aps):
    from contextlib import ExitStack
    import os
    ctx = ExitStack()
    _PH = int(os.environ.get("KERNEL_PHASES", "4"))

    persist = ctx.enter_context(tc.tile_pool(name="persist", bufs=1))
    ps = ctx.enter_context(tc.tile_pool(name="ps", bufs=6, space="PSUM"))
    # right-side pools released after the q projection
    ctxR = ExitStack()
    hidp = ctxR.enter_context(tc.tile_pool(name="hidp", bufs=1, side="right"))
    wpool = ctxR.enter_context(tc.tile_pool(name="wstream", bufs=8, side="right"))
    epool = ctxR.enter_context(tc.tile_pool(name="evac", bufs=2, side="right"))

    # ---------------- persistent tiles
    hidT8 = hidp.tile([128, 16, 2, KVG_ROWS], F8, tag="hidT8")
    bqb = hidp.tile([128, HD], BF16, tag="bqb")
    bob = persist.tile([128, HD], BF16, tag="bob")
    q_sb = persist.tile([128, 4, HD], BF16, tag="q_sb")
    # q8: fp8 q (16 head blocks) + 16 additive-mask blocks (DoubleRow halves)
    q8 = persist.tile([128, 32, QROWS], F8, tag="q8")
    kv_sb = persist.tile([128, NKV, 2 * D], BF16, tag="kv_sb")
    kvg_sb = persist.tile([G, 2 * D], BF16, tag="kvg_sb")
    kTm = persist.tile([128, NKV, 2, 128], F8, tag="kTm")
    kgTm = persist.tile([128, 2, G], F8, tag="kgTm")
    wkv8 = persist.tile([128, 16, 2, 2 * D], F8, tag="wkv8")
    wo_sb = persist.tile([128, H, HD], BF16, tag="wo_sb")
    i8_sb = persist.tile([128, 128], F8, tag="i8")
    ib16 = persist.tile([128, 128], BF16, tag="ib16")
    ones_c64 = persist.tile([128, 1], BF16, tag="ones_c64")  # 64.0 column
    ones_r = persist.tile([1, 128], BF16, tag="ones_r")   # row (K=1, M=128)
    pk128 = persist.tile([128, 8 * HALF + 2 * NKV * HALF + NKV], F32,
                         tag="pk128")
    pk64 = persist.tile([G, 2 * HALF], F32, tag="pk64")
    pkb = persist.tile([1, 2 * D + HD], BF16, tag="pkb")
    cq_sb = pk128[:, 0:64].rearrange("p (so r) -> p so r", r=HALF)
    sq_sb = pk128[:, 64:128].rearrange("p (so r) -> p so r", r=HALF)
    ckv_sb = pk128[:, 128:224].rearrange("p (t r) -> p t r", r=HALF)
    skv_sb = pk128[:, 224:320].rearrange("p (t r) -> p t r", r=HALF)
    am_sb = pk128[:, 320:326]
    cg_sb = pk64[:, 0:HALF]
    sg_sb = pk64[:, HALF:2 * HALF]
    bkv_sb = pkb[:, 0:2 * D]
    bo_sb = pkb[:, 2 * D:2 * D + HD]

    # ---------------- small loads (Activation HWDGE queue, ordered by need)
    nc.gpsimd.memset(ones_c64[:], 64.0)
    nc.gpsimd.memset(ones_r[:], 1.0)
    for nm, t in (("pk128", pk128), ("pk64", pk64), ("pkb", pkb),
                  ("ib16", ib16), ("bqb", bqb), ("i8", i8_sb),
                  ("bob", bob), ("wkv8", wkv8)):
        nc.scalar.dma_start(out=t[:], in_=aps[nm])
    # additive mask blocks (g = 4..7 of q8's 8 groups)
    nc.scalar.dma_start(out=q8[:, 16:32, :], in_=aps["mask8"])
    # hidT8 lo planes on the Activation queue (KV-proj correction, ~30us in)
    for k2 in range(8):
        nc.scalar.dma_start(out=hidT8[:, 2 * k2:2 * k2 + 2, 0, :],
                            in_=aps["hidT8"][:, 2 * k2:2 * k2 + 2, 0, :])
    # identity carriers into the DoubleRow second halves of kTm / kgTm
    nc.vector.tensor_copy(
        kTm[:, :, 1, :],
        i8_sb[:].rearrange("p (o d) -> p o d", o=1).to_broadcast([128, NKV, 128]))
    nc.vector.tensor_copy(kgTm[:, 1, :], i8_sb[:, 0:G])

    # rotary (in-place, f32 temps): x1' = x1*c - x2*s ; x2' = x2*c + x1*s
    def rotary(x1, x2, c, s, shape, tag, eng=None):
        eng = eng or nc.vector
        t1 = epool.tile(shape, F32, tag=tag + "1")
        t2 = epool.tile(shape, F32, tag=tag + "2")
        eng.tensor_tensor(out=t1[:], in0=x1, in1=s, op=MUL)
        eng.tensor_tensor(out=t2[:], in0=x2, in1=s, op=MUL)
        eng.tensor_tensor(out=x1, in0=x1, in1=c, op=MUL)
        eng.tensor_tensor(out=x1, in0=x1, in1=t2[:], op=SUB)
        eng.tensor_tensor(out=x2, in0=x2, in1=c, op=MUL)
        eng.tensor_tensor(out=x2, in0=x2, in1=t1[:], op=ADD)

    # ---------------- q projection -> q_sb [128p, 4so, 2048c] (64-scaled)
    # processed in two so-halves so full-width wq8 chunks (large DMA
    # descriptors) fit 8 PSUM banks; chunks stay in SBUF for the 2nd half
    if _PH < 2:
        ctxR.close()
        ctx.close()
        return
    wq_ts = []
    for so in range(4):
        pq = [ps.tile([128, 512], F32, tag="ps", name=f"pq{so}_{i}")
              for i in range(4)]
        for k2 in range(8):
            if so == 0:
                nc.sync.dma_start(out=hidT8[:, 2 * k2:2 * k2 + 2, 1, :],
                                  in_=aps["hidT8"][:, 2 * k2:2 * k2 + 2, 1, :])
                wq_t = wpool.tile([128, 2, HD], F8, tag="wq_t",
                                  name=f"wq_t{k2}")
                nc.sync.dma_start(out=wq_t[:],
                                  in_=aps["wq8"][:, 2 * k2:2 * k2 + 2, :])
                wq_ts.append(wq_t)
            for cc in range(4):
                nc.tensor.matmul(pq[cc][:],
                                 hidT8[:, 2 * k2:2 * k2 + 2, 1,
                                       256 + so * 128:384 + so * 128],
                                 wq_ts[k2][:, :, cc * 512:(cc + 1) * 512],
                                 start=(k2 == 0), stop=(k2 == 7),
                                 perf_mode=DR)
        for cc in range(4):
            nc.vector.tensor_tensor(
                out=q_sb[:, so, cc * 512:(cc + 1) * 512],
                in0=pq[cc][:],
                in1=bqb[:, cc * 512:(cc + 1) * 512], op=ADD)
        # rotary (odd so on gpsimd, even on DVE: parallel streams),
        # then on-chip PE transposes into the fp8 q8 blocks
        qv = q_sb[:, so, :].rearrange("p (h d) -> p h d", d=D)
        c = cq_sb[:, so:so + 1, :].to_broadcast([128, H, HALF])
        s = sq_sb[:, so:so + 1, :].to_broadcast([128, H, HALF])
        rotary(qv[:, :, 0:HALF], qv[:, :, HALF:2 * HALF], c, s,
               [128, H, HALF], "rq",
               eng=(nc.vector if so % 2 == 0 else nc.gpsimd))
        for hh in range(2):
            ptf = ps.tile([128, 8, 128], BF16, tag="pst", bufs=2,
                          name=f"ptq{so}_{hh}")
            pt = ptf[:]
            for h in range(8):
                nc.tensor.transpose(
                    pt[:, h, :],
                    q_sb[:, so, (8 * hh + h) * 128:(8 * hh + h + 1) * 128],
                    ib16[:])
            if hh == 0:
                nc.scalar.activation(
                    q8[:, 8 * hh:8 * hh + 8, so * 128:(so + 1) * 128],
                    pt, COPY, scale=1.0 / WS)
            else:
                nc.vector.tensor_scalar_mul(
                    q8[:, 8 * hh:8 * hh + 8, so * 128:(so + 1) * 128],
                    pt, 1.0 / WS)

    # ---------------- kv projection (+ glob rows); k naive, v compensated
    for st in range(NKV + 1):
        m = 128 if st < NKV else G
        pkv = ps.tile([128, 512], F32, tag="ps")     # main: 64*(k|v)
        pkc = ps.tile([128, 512], F32, tag="ps")     # corr: 16*64*v residue
        for k2 in range(8):
            nc.tensor.matmul(pkv[:m, :2 * D],
                             hidT8[:, 2 * k2:2 * k2 + 2, 1, st * 128:st * 128 + m],
                             wkv8[:, 2 * k2:2 * k2 + 2, 0, :],
                             start=(k2 == 0), stop=False, perf_mode=DR)
        nc.tensor.matmul(pkv[:m, :2 * D], ones_r[:, :m], bkv_sb[:],
                         start=False, stop=True)
        for kt in range(16):
            nc.tensor.matmul(pkc[:m, :D],
                             hidT8[:, kt, :, st * 128:st * 128 + m],
                             wkv8[:, kt, :, D:2 * D],
                             start=(kt == 0), stop=(kt == 15), perf_mode=DR)
        dst = kv_sb[:, st, :] if st < NKV else kvg_sb[:]
        vcorr = epool.tile([128, D], F32, tag="vcorr")
        nc.scalar.activation(vcorr[:m], pkc[:m, :D], COPY, scale=1.0 / 16.0)
        nc.scalar.copy(dst[:m, 0:D], pkv[:m, 0:D])
        nc.vector.tensor_tensor(out=dst[:m, D:2 * D], in0=pkv[:m, D:2 * D],
                                in1=vcorr[:m], op=ADD)

    rotary(kv_sb[:, :, 0:HALF], kv_sb[:, :, HALF:2 * HALF],
           ckv_sb[:], skv_sb[:], [128, NKV, HALF], "rkv", eng=nc.gpsimd)
    rotary(kvg_sb[:, 0:HALF], kvg_sb[:, HALF:2 * HALF],
           cg_sb[:], sg_sb[:], [G, HALF], "rg", eng=nc.gpsimd)

    # kT via PE transposes straight into the fp8 DoubleRow layout
    ptkf = ps.tile([128, 8, 128], BF16, tag="pst", bufs=2, name="ptk")
    ptk = ptkf[:]
    for st in range(NKV):
        nc.tensor.transpose(ptk[:, st, :], kv_sb[:, st, 0:128], ib16[:])
    nc.scalar.activation(kTm[:, :, 0, :], ptk[:, 0:NKV, :], COPY,
                         scale=1.0 / WS)
    ptgf = ps.tile([128, 8, 128], BF16, tag="pst", bufs=2, name="ptg")
    ptg = ptgf[:, 0, :]
    nc.tensor.transpose(ptg[:, 0:G], kvg_sb[:, 0:128], ib16[0:G, 0:G])
    nc.scalar.activation(kgTm[:, 0, :], ptg[:, 0:G], COPY, scale=1.0 / WS)

    # wo loads: gated on q8 readiness so the DMA engines stay clear for the
    # projection-phase critical path; they stream during early attention.
    for hg in range(4):
        nc.gpsimd.tensor_copy(wo_sb[0:1, 0, hg * 512:hg * 512 + 1],
                              q8[0:1, 4 * hg, 0:1])
        nc.sync.dma_start(
            out=wo_sb[:, :, hg * 512:(hg + 1) * 512],
            in_=aps["wo"].rearrange("(h p) n -> p h n", p=128)
                [:, :, hg * 512:(hg + 1) * 512])

    ctxR.close()

    # ---------------- fused attention + out-projection, per block L
    wexp = ctx.enter_context(tc.tile_pool(name="wexp", bufs=8))
    rzp = ctx.enter_context(tc.tile_pool(name="rzp", bufs=4))
    opool = ctx.enter_context(tc.tile_pool(name="opool", bufs=2))
    if _PH < 3:
        ctx.close()
        return
    aT_tiles = [None] * 4
    aT8_tiles = [None] * 4
    # g-blocks of q8: 0-3 = q head groups, 4-6 = win masks t, 7 = glob mask
    q8v = q8[:].rearrange("p (g h) r -> p g h r", h=4)

    def scores(L, hg):
        """Emit the 4 DoubleRow score matmuls (half1 carries additive mask)."""
        st = {}
        for t in range(3):
            rhs = q8v[:, hg:5 + t:(4 + t - hg), :, L * 128:(L + 1) * 128]
            p_t = ps.tile([128, 512], F32, tag="ps", name=f"p_t{L}_{hg}_{t}")
            nc.tensor.matmul(p_t[:], kTm[:, L + t, :, :], rhs,
                             start=True, stop=True, perf_mode=DR)
            st[t] = p_t
        rhs = q8v[:, hg:8:(7 - hg), :, L * 128:(L + 1) * 128]
        p_g = ps.tile([128, 512], F32, tag="ps", name=f"p_g{L}_{hg}")
        nc.tensor.matmul(p_g[:G, :], kgTm[:], rhs, start=True, stop=True,
                         perf_mode=DR)
        st["g"] = p_g
        return st

    def post(L, hg, st, aT):
        """exp -> softmax sums -> AV -> normalized aT for (L, hg)."""
        w_t = []
        for t in range(3):
            w = wexp.tile([128, 512], BF16, tag="wexp", name=f"w{L}_{hg}_{t}")
            nc.scalar.activation(w[:], st[t][:], EXP,
                                 bias=am_sb[:, L + t:L + t + 1], scale=SCALE)
            w_t.append(w)
        w_g = wexp.tile([G, 512], BF16, tag="wexpg", bufs=4,
                        name=f"wg{L}_{hg}")
        nc.scalar.activation(w_g[:], st["g"][:G, :], EXP, scale=SCALE)

        # softmax denominator: 64z via ones-matmuls, broadcast, reciprocal
        # (the 64 cancels po's 64-scale in the aT multiply)
        pz = ps.tile([128, 512], F32, tag="ps", name=f"pz{L}_{hg}")
        for t in range(3):
            nc.tensor.matmul(pz[:1, :], ones_c64[:], w_t[t][:],
                             start=(t == 0), stop=False)
        nc.tensor.matmul(pz[:1, :], ones_c64[:G, :], w_g[:],
                         start=False, stop=True)
        zh = rzp.tile([1, 512], BF16, tag="z_sb", name=f"z{L}_{hg}")
        nc.vector.tensor_copy(zh[:], pz[:1, :])

        po = ps.tile([128, 512], F32, tag="ps", name=f"po{L}_{hg}")
        for t in range(3):
            nc.tensor.matmul(po[:], kv_sb[:, L + t, D:2 * D], w_t[t][:],
                             start=(t == 0), stop=False)
        nc.tensor.matmul(po[:], kvg_sb[:, D:2 * D], w_g[:],
                         start=False, stop=True)

        pzb = ps.tile([128, 512], F32, tag="ps", name=f"pzb{L}_{hg}")
        nc.tensor.matmul(pzb[:], ones_r[:], zh[:], start=True, stop=True)
        rzb = rzp.tile([128, 512], F32, tag="rzb", name=f"rzb{L}_{hg}")
        nc.vector.reciprocal(out=rzb[:], in_=pzb[:])
        nc.vector.tensor_tensor(
            out=aT[:, 4 * hg:4 * hg + 4, :],
            in0=po[:].rearrange("p (h s) -> p h s", s=128),
            in1=rzb[:].rearrange("p (h s) -> p h s", s=128),
            op=MUL)

    # out-projection for block Lp, one output-column chunk ncn per call
    oproj_state = {}

    def oproj_seg(Lp, ncn):
        aT = aT_tiles[Lp]
        po2 = ps.tile([128, 512], F32, tag="ps", name=f"po2_{Lp}_{ncn}")
        for h in range(16):
            nc.tensor.matmul(po2[:], aT[:, h, :],
                             wo_sb[:, h, ncn * 512:(ncn + 1) * 512],
                             start=(h == 0), stop=(h == 15))
        o_sb = oproj_state.setdefault(
            Lp, opool.tile([128, HD], BF16, tag="o_sb", name=f"o_sb{Lp}"))
        nc.vector.tensor_tensor(
            out=o_sb[:, ncn * 512:(ncn + 1) * 512], in0=po2[:],
            in1=bob[:, ncn * 512:(ncn + 1) * 512], op=ADD)
        if ncn == 3:
            nc.sync.dma_start(out=aps["out"][Lp * 128:(Lp + 1) * 128, :],
                              in_=o_sb[:])

    # software pipeline: scores(hg+1) issued before post(hg); out-proj of
    # block L-1 interleaved between posts. wo streams in n-column chunks on
    # the sync queue during attention L0 (keeps it off the startup DMA path).
    for L in range(4):
        aT_tiles[L] = wexp.tile([128, H, 128], BF16, tag="aT", bufs=2,
                                name=f"aT{L}")
        st_prev = scores(L, 0)
        for hg in range(4):
            st_next = scores(L, hg + 1) if hg < 3 else None
            post(L, hg, st_prev, aT_tiles[L])
            st_prev = st_next
            if _PH >= 4 and L >= 1:
                oproj_seg(L - 1, hg)
    if _PH >= 4:
        for ncn in range(4):
            oproj_seg(3, ncn)

    ctx.close()


# ------------------------------------------------------------------ host ----

_NC_CACHE = None


def _get_nc():
    global _NC_CACHE
    if _NC_CACHE is None:
        _NC_CACHE = build_nc()
    return _NC_CACHE


def _f8(x):
    return np.asarray(x, np.float32).astype(F8NP)


def _f8_pair(x):
    """(hi, lo) with x ~= hi + lo/16, both fp8."""
    hi = _f8(x)
    lo = _f8((np.asarray(x, np.float32) - hi.astype(np.float32)) * 16.0)
    return hi, lo


def make_in_maps(hidden_states, attention_mask, glob_idx, W_qkv, b_qkv, W_o, b_o):
    bf = ml_dtypes.bfloat16
    hidden_states = np.asarray(hidden_states, np.float32)
    attention_mask = np.asarray(attention_mask, np.float32)
    glob_idx = np.asarray(glob_idx)
    W_qkv = np.asarray(W_qkv, np.float32)
    b_qkv = np.asarray(b_qkv, np.float32)
    W_o = np.asarray(W_o, np.float32)
    b_o = np.asarray(b_o, np.float32)

    w3 = W_qkv.reshape(HD, H, 3 * D)
    wq = np.ascontiguousarray(w3[:, :, :D].reshape(HD, HD))
    wkv = np.concatenate([w3[:, :, D:2 * D].mean(axis=1),
                          w3[:, :, 2 * D:].mean(axis=1)], axis=1)
    # feature-major fp8 layouts, 64-scaled
    wq8 = _f8((WS * wq).reshape(16, 128, HD).transpose(1, 0, 2))
    wkv_hi, wkv_lo = _f8_pair((WS * wkv).reshape(16, 128, 2 * D))
    wkv8 = np.stack([wkv_hi, wkv_lo], axis=2).transpose(1, 0, 2, 3)
    wkv8 = np.ascontiguousarray(wkv8)      # [128, 16, 2(hi,lo), 256]

    b3 = b_qkv.reshape(H, 3 * D)
    bq = np.ascontiguousarray(b3[:, :D].reshape(1, HD)).astype(np.float32)
    bqb = np.ascontiguousarray(np.broadcast_to(WS * bq, (128, HD))).astype(bf)
    bob = np.ascontiguousarray(np.broadcast_to(b_o[None, :], (128, HD))
                               ).astype(bf)
    bkv = np.concatenate([b3[:, D:2 * D].mean(axis=0),
                          b3[:, 2 * D:].mean(axis=0)])[None, :] * WS
    bo = b_o[None, :]
    pkb = np.concatenate([bkv, bo], axis=1).astype(bf)
    wo = W_o.astype(bf)

    inv_freq = 1.0 / (BASE ** (np.arange(0, ROT, 2, dtype=np.float32) / ROT))
    freqs = np.arange(S, dtype=np.float32)[:, None] * inv_freq[None, :]  # [S,16]
    cos_all = np.cos(freqs).astype(np.float32)
    sin_all = np.sin(freqs).astype(np.float32)

    in_maps = []
    for c in range(NCORES):
        b, q = divmod(c, 4)
        t0 = 4 * q - 2
        tiles = [max(0, t0 + i) for i in range(NKV)]       # clipped content
        intended = [t0 + i for i in range(NKV)]
        kv_rows = np.concatenate([np.arange(t * 128, t * 128 + 128)
                                  for t in tiles])
        g_rows = glob_idx[b].astype(np.int64)
        rows = np.concatenate([kv_rows, g_rows])
        hid_c = np.ascontiguousarray(hidden_states[b][rows])   # [832, 2048]
        # transpose to [128 fsub, 16 ftile, rows], fp8 (lo, hi) planes
        hidT = hid_c.T.reshape(16, 128, KVG_ROWS).transpose(1, 0, 2)
        h_hi, h_lo = _f8_pair(hidT)
        hidT8 = np.ascontiguousarray(np.stack([h_lo, h_hi], axis=2))

        q_rows = np.arange(QROWS * q, QROWS * (q + 1))
        cos_q = cos_all[q_rows].reshape(4, 128, HALF).transpose(1, 0, 2).copy()
        sin_q = sin_all[q_rows].reshape(4, 128, HALF).transpose(1, 0, 2).copy()
        cos_kv = cos_all[kv_rows].reshape(NKV, 128, HALF).transpose(1, 0, 2).copy()
        sin_kv = sin_all[kv_rows].reshape(NKV, 128, HALF).transpose(1, 0, 2).copy()
        cos_g = cos_all[g_rows].copy()
        sin_g = sin_all[g_rows].copy()

        am = attention_mask[b, 0, 0]                        # [S]
        am_loc = am[kv_rows].reshape(NKV, 128).T.copy()     # [128, NKV]

        # additive fp8 masks, folded into the DoubleRow score matmuls:
        # score += 240 * m8 where m8 = -240 for invalid -> -57600 pre-scale.
        # layout [key-p, block(t0,t1,t2,glob), 4h replicated, 4L*128 rows]
        mask8 = np.full((128, 4, 4, 512), -240.0, np.float32)
        for L in range(4):
            rows_glb = QROWS * q + L * 128 + np.arange(128)
            for t in range(3):
                it = intended[L + t]
                if it < 0:
                    continue
                key_pos = it * 128 + np.arange(128)
                valid = (key_pos[:, None] <= rows_glb[None, :]) & \
                        (key_pos[:, None] >= rows_glb[None, :] - (WIN - 1))
                mask8[:, t, :, L * 128:(L + 1) * 128] = \
                    np.where(valid, 0.0, -240.0)[:, None, :]
        # glob (block 3): row >= WIN and glob_idx < row - WIN; key rows 64-127
        # are dead (identity carrier is zero there)
        rows_glb = QROWS * q + np.arange(QROWS)
        validg = ((rows_glb[None, :] >= WIN) &
                  (g_rows[:, None] < rows_glb[None, :] - WIN))
        mask8[:G, 3, :, :] = np.where(validg, 0.0, -240.0)[:, None, :]
        mask8 = mask8.reshape(128, 16, 512)

        i8 = (240.0 * np.eye(128, dtype=np.float32))
        ib16 = np.eye(128, dtype=np.float32).astype(bf)

        pk128 = np.concatenate(
            [cos_q.reshape(128, 64), sin_q.reshape(128, 64),
             cos_kv.reshape(128, 96), sin_kv.reshape(128, 96),
             am_loc], axis=1).astype(np.float32)
        pk64 = np.concatenate([cos_g, sin_g], axis=1).astype(np.float32)
        in_maps.append({
            "hidT8": hidT8,
            "wq8": wq8, "wkv8": wkv8, "wo": wo,
            "bqb": bqb, "bob": bob,
            "pk128": pk128, "pk64": pk64, "pkb": pkb,
            "i8": _f8(i8), "ib16": ib16, "mask8": _f8(mask8),
        })
    return in_maps


def kernel(hidden_states, attention_mask, glob_idx, W_qkv, b_qkv, W_o, b_o):
    nc = _get_nc()
    in_maps = make_in_maps(hidden_states, attention_mask, glob_idx,
                           W_qkv, b_qkv, W_o, b_o)
    res = run_bass_kernel_spmd(nc, in_maps, core_ids=list(range(NCORES)))
    out = np.empty((B, S, HD), np.float32)
    for c in range(NCORES):
        b, q = divmod(c, 4)
        out[b, QROWS * q:QROWS * (q + 1), :] = \
            res.results[c]["out"].astype(np.float32)
    return out


# revision 69
# speedup vs baseline: 1.0508x; 1.0200x over previous
"""Trainium2 Bass kernel: Longformer-style windowed attention with rotary,
head-averaged K/V (step_attn), fused QKV/out projections.

Sharding: 8 cores = (batch 2) x (sequence-quarter 4). Each core computes its
512 output rows for all 16 heads. No collectives: the windowed attention for
a 512-row quarter only needs 6 key-tiles (128 rows each) of the head-averaged
K/V plus the 64 global-token rows, all of which the core computes itself from
host-sliced hidden-state rows. Head-averaging of K/V commutes with rotary and
with the (linear) projection, so the K/V-mean projection weights are folded on
host to [2048, 256].

fp8 strategy (DoubleRow matmuls, 2 k-tiles of 128 per instruction at 0.5
cycles/row): error-attenuated paths (Q proj, K proj, QK scores) run naive fp8;
the V projection is hi+lo compensated (residual term via one DoubleRow matmul
per k-tile pairing hid_lo*w_hi + hi*lo); attention-value matmul, softmax sums
and out-projection stay bf16. Weights are host-scaled by 64 to clear the fp8
subnormal range; the 1/64 folds into downstream constant scales.
"""

import sys

for _p in ("/opt/trn_rl_repo", "/root/.axon_site/_ro/trn_rl_repo"):
    if _p not in sys.path:
        sys.path.append(_p)

import numpy as np
import ml_dtypes

import concourse.bass as bass
import concourse.tile as tile
from concourse import bacc
from concourse import bass_isa
import concourse.mybir as mybir
from concourse.bass_utils import run_bass_kernel_spmd

F32 = mybir.dt.float32
BF16 = mybir.dt.bfloat16
F8 = mybir.dt.float8e4
F8NP = ml_dtypes.float8_e4m3
DR = mybir.MatmulPerfMode.DoubleRow
MUL = mybir.AluOpType.mult
ADD = mybir.AluOpType.add
SUB = mybir.AluOpType.subtract
COPY = mybir.ActivationFunctionType.Copy
EXP = mybir.ActivationFunctionType.Exp

H = 16
D = 128
ROT = 32
HALF = 16  # ROT // 2
WIN = 256
G = 64
BASE = 10000.0
S = 2048
HD = H * D
B = 2
NCORES = 8
QROWS = 512          # rows per core
NKV = 6              # kv key-tiles per core
KVG_ROWS = NKV * 128 + G  # 832
SCALE = 1.0 / float(np.sqrt(np.float32(D)))
WS = 64.0            # host weight scale (fp8 subnormal avoidance)


# ---------------------------------------------------------------- device ----

def build_nc():
    nc = bacc.Bacc("TRN2", target_bir_lowering=False, debug=False,
                   num_devices=NCORES)

    aps = {}
    def inp(name, shape, dt):
        aps[name] = nc.dram_tensor(name, shape, dt, kind="ExternalInput").ap()

    # hidT8: transposed hidden states, fp8, planes (lo, hi) per k-tile
    inp("hidT8", [128, 16, 2, KVG_ROWS], F8)
    inp("wq8", [128, 16, HD], F8)            # 64*Wq, feature-major, naive
    inp("wkv8", [128, 16, 2, 2 * D], F8)     # 64*Wkv, planes (hi, lo16)
    inp("wo", [HD, HD], BF16)
    inp("bqb", [128, HD], BF16)              # 64*b_q broadcast to partitions
    inp("bob", [128, HD], BF16)              # b_o broadcast to partitions
    inp("pk128", [128, 8 * HALF + 2 * NKV * HALF + NKV], F32)
    inp("pk64", [G, 2 * HALF], F32)
    inp("pkb", [1, 2 * D + HD], BF16)        # 64*b_kv | b_o
    inp("i8", [128, 128], F8)                # 240 * identity (mask carrier)
    inp("ib16", [128, 128], BF16)            # identity (PE transposes)
    inp("mask8", [128, 16, 512], F8)         # -240*!valid: (3 win t + glob)x4h
    aps["out"] = nc.dram_tensor("out", [QROWS, HD], BF16,
                                kind="ExternalOutput").ap()

    with tile.TileContext(nc) as tc:
        _build_tile(nc, tc, aps)
    nc.compile()
    return nc


def _build_tile(nc, tc, aps):
    from contextlib import ExitStack
    import os
    ctx = ExitStack()
    _PH = int(os.environ.get("KERNEL_PHASES", "4"))

    persist = ctx.enter_context(tc.tile_pool(name="persist", bufs=1))
    ps = ctx.enter_context(tc.tile_pool(name="ps", bufs=7, space="PSUM"))
    # right-side pools released after the q projection
    ctxR = ExitStack()
    hidp = ctxR.enter_context(tc.tile_pool(name="hidp", bufs=1, side="right"))
    wpool = ctxR.enter_context(tc.tile_pool(name="wstream", bufs=8, side="right"))
    epool = ctxR.enter_context(tc.tile_pool(name="evac", bufs=2, side="right"))

    # ---------------- persistent tiles
    hidT8 = hidp.tile([128, 16, 2, KVG_ROWS], F8, tag="hidT8")
    bqb = hidp.tile([128, HD], BF16, tag="bqb")
    bob = persist.tile([128, HD], BF16, tag="bob")
    q_sb = persist.tile([128, 4, HD], BF16, tag="q_sb")
    # q8: fp8 q (16 head blocks) + 16 additive-mask blocks (DoubleRow halves)
    q8 = persist.tile([128, 32, QROWS], F8, tag="q8")
    kv_sb = persist.tile([128, NKV, 2 * D], BF16, tag="kv_sb")
    kvg_sb = persist.tile([G, 2 * D], BF16, tag="kvg_sb")
    kTm = persist.tile([128, NKV, 2, 128], F8, tag="kTm")
    kgTm = persist.tile([128, 2, G], F8, tag="kgTm")
    wkv8 = persist.tile([128, 16, 2, 2 * D], F8, tag="wkv8")
    wo_sb = persist.tile([128, H, HD], BF16, tag="wo_sb")
    i8_sb = persist.tile([128, 128], F8, tag="i8")
    ib16 = persist.tile([128, 128], BF16, tag="ib16")
    ones_c64 = persist.tile([128, 1], BF16, tag="ones_c64")  # 64.0 column
    ones_r = persist.tile([1, 128], BF16, tag="ones_r")   # row (K=1, M=128)
    pk128 = persist.tile([128, 8 * HALF + 2 * NKV * HALF + NKV], F32,
                         tag="pk128")
    pk64 = persist.tile([G, 2 * HALF], F32, tag="pk64")
    pkb = persist.tile([1, 2 * D + HD], BF16, tag="pkb")
    cq_sb = pk128[:, 0:64].rearrange("p (so r) -> p so r", r=HALF)
    sq_sb = pk128[:, 64:128].rearrange("p (so r) -> p so r", r=HALF)
    ckv_sb = pk128[:, 128:224].rearrange("p (t r) -> p t r", r=HALF)
    skv_sb = pk128[:, 224:320].rearrange("p (t r) -> p t r", r=HALF)
    am_sb = pk128[:, 320:326]
    cg_sb = pk64[:, 0:HALF]
    sg_sb = pk64[:, HALF:2 * HALF]
    bkv_sb = pkb[:, 0:2 * D]
    bo_sb = pkb[:, 2 * D:2 * D + HD]

    # ---------------- small loads (Activation HWDGE queue, ordered by need)
    nc.gpsimd.memset(ones_c64[:], 64.0)
    nc.gpsimd.memset(ones_r[:], 1.0)
    for nm, t in (("pk128", pk128), ("pk64", pk64), ("pkb", pkb),
                  ("ib16", ib16), ("bqb", bqb), ("i8", i8_sb),
                  ("bob", bob), ("wkv8", wkv8)):
        nc.scalar.dma_start(out=t[:], in_=aps[nm])
    # additive mask blocks (g = 4..7 of q8's 8 groups)
    nc.scalar.dma_start(out=q8[:, 16:32, :], in_=aps["mask8"])
    # hidT8 lo planes on the Activation queue (KV-proj correction, ~30us in)
    for k2 in range(8):
        nc.scalar.dma_start(out=hidT8[:, 2 * k2:2 * k2 + 2, 0, :],
                            in_=aps["hidT8"][:, 2 * k2:2 * k2 + 2, 0, :])
    # identity carriers into the DoubleRow second halves of kTm / kgTm
    nc.vector.tensor_copy(
        kTm[:, :, 1, :],
        i8_sb[:].rearrange("p (o d) -> p o d", o=1).to_broadcast([128, NKV, 128]))
    nc.vector.tensor_copy(kgTm[:, 1, :], i8_sb[:, 0:G])

    # rotary (in-place, f32 temps): x1' = x1*c - x2*s ; x2' = x2*c + x1*s
    def rotary(x1, x2, c, s, shape, tag, eng=None):
        eng = eng or nc.vector
        t1 = epool.tile(shape, F32, tag=tag + "1")
        t2 = epool.tile(shape, F32, tag=tag + "2")
        eng.tensor_tensor(out=t1[:], in0=x1, in1=s, op=MUL)
        eng.tensor_tensor(out=t2[:], in0=x2, in1=s, op=MUL)
        eng.tensor_tensor(out=x1, in0=x1, in1=c, op=MUL)
        eng.tensor_tensor(out=x1, in0=x1, in1=t2[:], op=SUB)
        eng.tensor_tensor(out=x2, in0=x2, in1=c, op=MUL)
        eng.tensor_tensor(out=x2, in0=x2, in1=t1[:], op=ADD)

    # ---------------- q projection -> q_sb [128p, 4so, 2048c] (64-scaled)
    # processed in two so-halves so full-width wq8 chunks (large DMA
    # descriptors) fit 8 PSUM banks; chunks stay in SBUF for the 2nd half
    if _PH < 2:
        ctxR.close()
        ctx.close()
        return
    wq_ts = []
    for so in range(4):
        pq = [ps.tile([128, 512], F32, tag="ps", name=f"pq{so}_{i}")
              for i in range(4)]
        for k2 in range(8):
            if so == 0:
                nc.sync.dma_start(out=hidT8[:, 2 * k2:2 * k2 + 2, 1, :],
                                  in_=aps["hidT8"][:, 2 * k2:2 * k2 + 2, 1, :])
                wq_t = wpool.tile([128, 2, HD], F8, tag="wq_t",
                                  name=f"wq_t{k2}")
                nc.sync.dma_start(out=wq_t[:],
                                  in_=aps["wq8"][:, 2 * k2:2 * k2 + 2, :])
                wq_ts.append(wq_t)
            for cc in range(4):
                nc.tensor.matmul(pq[cc][:],
                                 hidT8[:, 2 * k2:2 * k2 + 2, 1,
                                       256 + so * 128:384 + so * 128],
                                 wq_ts[k2][:, :, cc * 512:(cc + 1) * 512],
                                 start=(k2 == 0), stop=(k2 == 7),
                                 perf_mode=DR)
        for cc in range(4):
            nc.vector.tensor_tensor(
                out=q_sb[:, so, cc * 512:(cc + 1) * 512],
                in0=pq[cc][:],
                in1=bqb[:, cc * 512:(cc + 1) * 512], op=ADD)
        # rotary (odd so on gpsimd, even on DVE: parallel streams),
        # then on-chip PE transposes into the fp8 q8 blocks
        qv = q_sb[:, so, :].rearrange("p (h d) -> p h d", d=D)
        c = cq_sb[:, so:so + 1, :].to_broadcast([128, H, HALF])
        s = sq_sb[:, so:so + 1, :].to_broadcast([128, H, HALF])
        rotary(qv[:, :, 0:HALF], qv[:, :, HALF:2 * HALF], c, s,
               [128, H, HALF], "rq",
               eng=(nc.vector if so % 2 == 0 else nc.gpsimd))
        for hh in range(2):
            ptf = ps.tile([128, 8, 128], BF16, tag="pst", bufs=1,
                          name=f"ptq{so}_{hh}")
            pt = ptf[:]
            for h in range(8):
                nc.tensor.transpose(
                    pt[:, h, :],
                    q_sb[:, so, (8 * hh + h) * 128:(8 * hh + h + 1) * 128],
                    ib16[:])
            if hh == 0:
                nc.scalar.activation(
                    q8[:, 8 * hh:8 * hh + 8, so * 128:(so + 1) * 128],
                    pt, COPY, scale=1.0 / WS)
            else:
                nc.vector.tensor_scalar_mul(
                    q8[:, 8 * hh:8 * hh + 8, so * 128:(so + 1) * 128],
                    pt, 1.0 / WS)

    # ---------------- kv projection (+ glob rows); k naive, v compensated
    for st in range(NKV + 1):
        m = 128 if st < NKV else G
        pkv = ps.tile([128, 512], F32, tag="ps")     # main: 64*(k|v)
        pkc = ps.tile([128, 512], F32, tag="ps")     # corr: 16*64*v residue
        for k2 in range(8):
            nc.tensor.matmul(pkv[:m, :2 * D],
                             hidT8[:, 2 * k2:2 * k2 + 2, 1, st * 128:st * 128 + m],
                             wkv8[:, 2 * k2:2 * k2 + 2, 0, :],
                             start=(k2 == 0), stop=False, perf_mode=DR)
        nc.tensor.matmul(pkv[:m, :2 * D], ones_r[:, :m], bkv_sb[:],
                         start=False, stop=True)
        for kt in range(16):
            nc.tensor.matmul(pkc[:m, :D],
                             hidT8[:, kt, :, st * 128:st * 128 + m],
                             wkv8[:, kt, :, D:2 * D],
                             start=(kt == 0), stop=(kt == 15), perf_mode=DR)
        dst = kv_sb[:, st, :] if st < NKV else kvg_sb[:]
        vcorr = epool.tile([128, D], F32, tag="vcorr")
        nc.scalar.activation(vcorr[:m], pkc[:m, :D], COPY, scale=1.0 / 16.0)
        nc.scalar.copy(dst[:m, 0:D], pkv[:m, 0:D])
        nc.vector.tensor_tensor(out=dst[:m, D:2 * D], in0=pkv[:m, D:2 * D],
                                in1=vcorr[:m], op=ADD)

    rotary(kv_sb[:, :, 0:HALF], kv_sb[:, :, HALF:2 * HALF],
           ckv_sb[:], skv_sb[:], [128, NKV, HALF], "rkv", eng=nc.gpsimd)
    rotary(kvg_sb[:, 0:HALF], kvg_sb[:, HALF:2 * HALF],
           cg_sb[:], sg_sb[:], [G, HALF], "rg", eng=nc.gpsimd)

    # kT via PE transposes straight into the fp8 DoubleRow layout
    ptkf = ps.tile([128, 8, 128], BF16, tag="pst", bufs=1, name="ptk")
    ptk = ptkf[:]
    for st in range(NKV):
        nc.tensor.transpose(ptk[:, st, :], kv_sb[:, st, 0:128], ib16[:])
    nc.scalar.activation(kTm[:, :, 0, :], ptk[:, 0:NKV, :], COPY,
                         scale=1.0 / WS)
    ptgf = ps.tile([128, 8, 128], BF16, tag="pst", bufs=1, name="ptg")
    ptg = ptgf[:, 0, :]
    nc.tensor.transpose(ptg[:, 0:G], kvg_sb[:, 0:128], ib16[0:G, 0:G])
    nc.scalar.activation(kgTm[:, 0, :], ptg[:, 0:G], COPY, scale=1.0 / WS)

    # wo loads: gated on q8 readiness so the DMA engines stay clear for the
    # projection-phase critical path; they stream during early attention.
    for hg in range(4):
        nc.gpsimd.tensor_copy(wo_sb[0:1, 0, hg * 512:hg * 512 + 1],
                              q8[0:1, 4 * hg, 0:1])
        nc.sync.dma_start(
            out=wo_sb[:, :, hg * 512:(hg + 1) * 512],
            in_=aps["wo"].rearrange("(h p) n -> p h n", p=128)
                [:, :, hg * 512:(hg + 1) * 512])

    ctxR.close()

    # ---------------- fused attention + out-projection, per block L
    wexp = ctx.enter_context(tc.tile_pool(name="wexp", bufs=8))
    rzp = ctx.enter_context(tc.tile_pool(name="rzp", bufs=4))
    opool = ctx.enter_context(tc.tile_pool(name="opool", bufs=2))
    if _PH < 3:
        ctx.close()
        return
    aT_tiles = [None] * 4
    aT8_tiles = [None] * 4
    # g-blocks of q8: 0-3 = q head groups, 4-6 = win masks t, 7 = glob mask
    q8v = q8[:].rearrange("p (g h) r -> p g h r", h=4)

    def scores(L, hg):
        """Emit the 4 DoubleRow score matmuls (half1 carries additive mask)."""
        st = {}
        for t in range(3):
            rhs = q8v[:, hg:5 + t:(4 + t - hg), :, L * 128:(L + 1) * 128]
            p_t = ps.tile([128, 512], F32, tag="ps", name=f"p_t{L}_{hg}_{t}")
            nc.tensor.matmul(p_t[:], kTm[:, L + t, :, :], rhs,
                             start=True, stop=True, perf_mode=DR)
            st[t] = p_t
        rhs = q8v[:, hg:8:(7 - hg), :, L * 128:(L + 1) * 128]
        p_g = ps.tile([128, 512], F32, tag="ps", name=f"p_g{L}_{hg}")
        nc.tensor.matmul(p_g[:G, :], kgTm[:], rhs, start=True, stop=True,
                         perf_mode=DR)
        st["g"] = p_g
        return st

    def post(L, hg, st, aT):
        """exp -> softmax sums -> AV -> normalized aT for (L, hg)."""
        w_t = []
        for t in range(3):
            w = wexp.tile([128, 512], BF16, tag="wexp", name=f"w{L}_{hg}_{t}")
            nc.scalar.activation(w[:], st[t][:], EXP,
                                 bias=am_sb[:, L + t:L + t + 1], scale=SCALE)
            w_t.append(w)
        w_g = wexp.tile([G, 512], BF16, tag="wexpg", bufs=4,
                        name=f"wg{L}_{hg}")
        nc.scalar.activation(w_g[:], st["g"][:G, :], EXP, scale=SCALE)

        # softmax denominator: 64z via ones-matmuls, broadcast, reciprocal
        # (the 64 cancels po's 64-scale in the aT multiply)
        pz = ps.tile([128, 512], F32, tag="ps", name=f"pz{L}_{hg}")
        for t in range(3):
            nc.tensor.matmul(pz[:1, :], ones_c64[:], w_t[t][:],
                             start=(t == 0), stop=False)
        nc.tensor.matmul(pz[:1, :], ones_c64[:G, :], w_g[:],
                         start=False, stop=True)
        zh = rzp.tile([1, 512], BF16, tag="z_sb", name=f"z{L}_{hg}")
        nc.vector.tensor_copy(zh[:], pz[:1, :])

        po = ps.tile([128, 512], F32, tag="ps", name=f"po{L}_{hg}")
        for t in range(3):
            nc.tensor.matmul(po[:], kv_sb[:, L + t, D:2 * D], w_t[t][:],
                             start=(t == 0), stop=False)
        nc.tensor.matmul(po[:], kvg_sb[:, D:2 * D], w_g[:],
                         start=False, stop=True)

        pzb = ps.tile([128, 512], F32, tag="ps", name=f"pzb{L}_{hg}")
        nc.tensor.matmul(pzb[:], ones_r[:], zh[:], start=True, stop=True)
        rzb = rzp.tile([128, 512], F32, tag="rzb", name=f"rzb{L}_{hg}")
        nc.vector.reciprocal(out=rzb[:], in_=pzb[:])
        nc.vector.tensor_tensor(
            out=aT[:, 4 * hg:4 * hg + 4, :],
            in0=po[:].rearrange("p (h s) -> p h s", s=128),
            in1=rzb[:].rearrange("p (h s) -> p h s", s=128),
            op=MUL)

    # out-projection for block Lp, one output-column chunk ncn per call
    oproj_state = {}

    def oproj_seg(Lp, ncn):
        aT = aT_tiles[Lp]
        po2 = ps.tile([128, 512], F32, tag="ps", name=f"po2_{Lp}_{ncn}")
        for h in range(16):
            nc.tensor.matmul(po2[:], aT[:, h, :],
                             wo_sb[:, h, ncn * 512:(ncn + 1) * 512],
                             start=(h == 0), stop=(h == 15))
        o_sb = oproj_state.setdefault(
            Lp, opool.tile([128, HD], BF16, tag="o_sb", name=f"o_sb{Lp}"))
        nc.vector.tensor_tensor(
            out=o_sb[:, ncn * 512:(ncn + 1) * 512], in0=po2[:],
            in1=bob[:, ncn * 512:(ncn + 1) * 512], op=ADD)
        if ncn == 3:
            nc.sync.dma_start(out=aps["out"][Lp * 128:(Lp + 1) * 128, :],
                              in_=o_sb[:])

    # software pipeline: scores(hg+1) issued before post(hg); out-proj of
    # block L-1 interleaved between posts. wo streams in n-column chunks on
    # the sync queue during attention L0 (keeps it off the startup DMA path).
    for L in range(4):
        aT_tiles[L] = wexp.tile([128, H, 128], BF16, tag="aT", bufs=2,
                                name=f"aT{L}")
        st_prev = scores(L, 0)
        for hg in range(4):
            st_next = scores(L, hg + 1) if hg < 3 else None
            post(L, hg, st_prev, aT_tiles[L])
            st_prev = st_next
            if _PH >= 4 and L >= 1:
                oproj_seg(L - 1, hg)
    if _PH >= 4:
        for ncn in range(4):
            oproj_seg(3, ncn)

    ctx.close()


# ------------------------------------------------------------------ host ----

_NC_CACHE = None


def _get_nc():
    global _NC_CACHE
    if _NC_CACHE is None:
        _NC_CACHE = build_nc()
    return _NC_CACHE


def _f8(x):
    return np.asarray(x, np.float32).astype(F8NP)


def _f8_pair(x):
    """(hi, lo) with x ~= hi + lo/16, both fp8."""
    hi = _f8(x)
    lo = _f8((np.asarray(x, np.float32) - hi.astype(np.float32)) * 16.0)
    return hi, lo


def make_in_maps(hidden_states, attention_mask, glob_idx, W_qkv, b_qkv, W_o, b_o):
    bf = ml_dtypes.bfloat16
    hidden_states = np.asarray(hidden_states, np.float32)
    attention_mask = np.asarray(attention_mask, np.float32)
    glob_idx = np.asarray(glob_idx)
    W_qkv = np.asarray(W_qkv, np.float32)
    b_qkv = np.asarray(b_qkv, np.float32)
    W_o = np.asarray(W_o, np.float32)
    b_o = np.asarray(b_o, np.float32)

    w3 = W_qkv.reshape(HD, H, 3 * D)
    wq = np.ascontiguousarray(w3[:, :, :D].reshape(HD, HD))
    wkv = np.concatenate([w3[:, :, D:2 * D].mean(axis=1),
                          w3[:, :, 2 * D:].mean(axis=1)], axis=1)
    # feature-major fp8 layouts, 64-scaled
    wq8 = _f8((WS * wq).reshape(16, 128, HD).transpose(1, 0, 2))
    wkv_hi, wkv_lo = _f8_pair((WS * wkv).reshape(16, 128, 2 * D))
    wkv8 = np.stack([wkv_hi, wkv_lo], axis=2).transpose(1, 0, 2, 3)
    wkv8 = np.ascontiguousarray(wkv8)      # [128, 16, 2(hi,lo), 256]

    b3 = b_qkv.reshape(H, 3 * D)
    bq = np.ascontiguousarray(b3[:, :D].reshape(1, HD)).astype(np.float32)
    bqb = np.ascontiguousarray(np.broadcast_to(WS * bq, (128, HD))).astype(bf)
    bob = np.ascontiguousarray(np.broadcast_to(b_o[None, :], (128, HD))
                               ).astype(bf)
    bkv = np.concatenate([b3[:, D:2 * D].mean(axis=0),
                          b3[:, 2 * D:].mean(axis=0)])[None, :] * WS
    bo = b_o[None, :]
    pkb = np.concatenate([bkv, bo], axis=1).astype(bf)
    wo = W_o.astype(bf)

    inv_freq = 1.0 / (BASE ** (np.arange(0, ROT, 2, dtype=np.float32) / ROT))
    freqs = np.arange(S, dtype=np.float32)[:, None] * inv_freq[None, :]  # [S,16]
    cos_all = np.cos(freqs).astype(np.float32)
    sin_all = np.sin(freqs).astype(np.float32)

    in_maps = []
    for c in range(NCORES):
        b, q = divmod(c, 4)
        t0 = 4 * q - 2
        tiles = [max(0, t0 + i) for i in range(NKV)]       # clipped content
        intended = [t0 + i for i in range(NKV)]
        kv_rows = np.concatenate([np.arange(t * 128, t * 128 + 128)
                                  for t in tiles])
        g_rows = glob_idx[b].astype(np.int64)
        rows = np.concatenate([kv_rows, g_rows])
        hid_c = np.ascontiguousarray(hidden_states[b][rows])   # [832, 2048]
        # transpose to [128 fsub, 16 ftile, rows], fp8 (lo, hi) planes
        hidT = hid_c.T.reshape(16, 128, KVG_ROWS).transpose(1, 0, 2)
        h_hi, h_lo = _f8_pair(hidT)
        hidT8 = np.ascontiguousarray(np.stack([h_lo, h_hi], axis=2))

        q_rows = np.arange(QROWS * q, QROWS * (q + 1))
        cos_q = cos_all[q_rows].reshape(4, 128, HALF).transpose(1, 0, 2).copy()
        sin_q = sin_all[q_rows].reshape(4, 128, HALF).transpose(1, 0, 2).copy()
        cos_kv = cos_all[kv_rows].reshape(NKV, 128, HALF).transpose(1, 0, 2).copy()
        sin_kv = sin_all[kv_rows].reshape(NKV, 128, HALF).transpose(1, 0, 2).copy()
        cos_g = cos_all[g_rows].copy()
        sin_g = sin_all[g_rows].copy()

        am = attention_mask[b, 0, 0]                        # [S]
        am_loc = am[kv_rows].reshape(NKV, 128).T.copy()     # [128, NKV]

        # additive fp8 masks, folded into the DoubleRow score matmuls:
        # score += 240 * m8 where m8 = -240 for invalid -> -57600 pre-scale.
        # layout [key-p, block(t0,t1,t2,glob), 4h replicated, 4L*128 rows]
        mask8 = np.full((128, 4, 4, 512), -240.0, np.float32)
        for L in range(4):
            rows_glb = QROWS * q + L * 128 + np.arange(128)
            for t in range(3):
                it = intended[L + t]
                if it < 0:
                    continue
                key_pos = it * 128 + np.arange(128)
                valid = (key_pos[:, None] <= rows_glb[None, :]) & \
                        (key_pos[:, None] >= rows_glb[None, :] - (WIN - 1))
                mask8[:, t, :, L * 128:(L + 1) * 128] = \
                    np.where(valid, 0.0, -240.0)[:, None, :]
        # glob (block 3): row >= WIN and glob_idx < row - WIN; key rows 64-127
        # are dead (identity carrier is zero there)
        rows_glb = QROWS * q + np.arange(QROWS)
        validg = ((rows_glb[None, :] >= WIN) &
                  (g_rows[:, None] < rows_glb[None, :] - WIN))
        mask8[:G, 3, :, :] = np.where(validg, 0.0, -240.0)[:, None, :]
        mask8 = mask8.reshape(128, 16, 512)

        i8 = (240.0 * np.eye(128, dtype=np.float32))
        ib16 = np.eye(128, dtype=np.float32).astype(bf)

        pk128 = np.concatenate(
            [cos_q.reshape(128, 64), sin_q.reshape(128, 64),
             cos_kv.reshape(128, 96), sin_kv.reshape(128, 96),
             am_loc], axis=1).astype(np.float32)
        pk64 = np.concatenate([cos_g, sin_g], axis=1).astype(np.float32)
        in_maps.append({
            "hidT8": hidT8,
            "wq8": wq8, "wkv8": wkv8, "wo": wo,
            "bqb": bqb, "bob": bob,
            "pk128": pk128, "pk64": pk64, "pkb": pkb,
            "i8": _f8(i8), "ib16": ib16, "mask8": _f8(mask8),
        })
    return in_maps


def kernel(hidden_states, attention_mask, glob_idx, W_qkv, b_qkv, W_o, b_o):
    nc = _get_nc()
    in_maps = make_in_maps(hidden_states, attention_mask, glob_idx,
                           W_qkv, b_qkv, W_o, b_o)
    res = run_bass_kernel_spmd(nc, in_maps, core_ids=list(range(NCORES)))
    out = np.empty((B, S, HD), np.float32)
    for c in range(NCORES):
        b, q = divmod(c, 4)
        out[b, QROWS * q:QROWS * (q + 1), :] = \
            res.results[c]["out"].astype(np.float32)
    return out
